# revision 1
# baseline (speedup 1.0000x reference)
"""Trainium2 Bass kernel: BFP (block-floating-point) activation quantization.

Reference semantics (input NCHW [32, 256, 56, 56] f32):
  per (batch, pixel), channels grouped in blocks of 32:
    maxabs = max |x| over the block
    e      = floor(log2(maxabs))          (guard zero blocks)
    s      = 2^(e-4)                      (5-bit mantissa, QMAX = 31)
    out    = clip(round_half_even(x / s), -31, 31) * s    (0 if maxabs == 0)

Implementation (bit-exact in fp32, validated against the reference):
  s0 = 2^e is extracted by masking the exponent bits of maxabs.  The whole
  round+clip+rescale collapses into one fused DVE op using magic-number
  rounding in the C = 1.5*2^23 * s domain:
      C  = s0 * 786432.0        (= 1.5*2^23 * 2^-4 * s0 = magic * s)
      m  = s0 * 1.9375          (= 31 * s)
      out = min(max(x + C, C - m), C + m) - C
  Every step is exact in fp32: the x + C addition performs the
  round-half-even at ULP = s, the clip bounds and the final subtraction are
  exact multiples of s in the same binade.  The outputs are +-q * 2^(e-4)
  with q <= 31 (5 significant bits), so they are exactly representable in
  bf16 — the backward transposes run in bf16 at half cost.

Layout: channels live on SBUF partitions after the natural NCHW DMA, but the
block reduction needs channels along the free dim, so tiles are transposed
through the (otherwise idle) tensor engine in 128x128 chunks, processed in
the pixel-on-partition layout, and transposed back.  The emission is
software-pipelined (forward transposes run one tile ahead) so the in-order
PE queue never head-of-line blocks on a tile's backward transposes, and
DMAs are split per tile with loads on the SP queue and stores on the ACT
queue so they overlap compute instead of bracketing it.

Sharding: batch 32 -> 4 per core across 8 NeuronCores; no cross-core comms.
"""

import numpy as np

import concourse.bass as bass
import concourse.mybir as mybir
from concourse import bacc, masks, tile
from concourse.bass_utils import run_bass_kernel_spmd

F32 = mybir.dt.float32
BF16 = mybir.dt.bfloat16
I32 = mybir.dt.int32

# ---------------------------------------------------------------------------
# Custom DVE op: the entire quantize in one 1x pass.
#   out = min(max(Src0 + Src1*C0, Src1*C0 - Src1*C1), Src1*C0 + Src1*C1) - Src1*C0
#   Src0 = x (pixel-major tile), Src1 = s0 = 2^e broadcast over the 32-chunk,
#   C0 = 786432.0, C1 = 1.9375.
# ---------------------------------------------------------------------------
_OP_NAME = "BFP_Q5_ANT"


def _bfp_q5_reference(in0, in1, s0, s1, imm2):
    in0 = np.asarray(in0, np.float32)
    in1 = np.asarray(in1, np.float32).reshape(in0.shape)
    c = (in1 * np.float32(s0)).astype(np.float32)
    m = (in1 * np.float32(s1)).astype(np.float32)
    u = (in0 + c).astype(np.float32)
    v = np.minimum(np.maximum(u, (c - m).astype(np.float32)),
                   (c + m).astype(np.float32)).astype(np.float32)
    return (v - c).astype(np.float32)


def _register_custom_op():
    import concourse.dve_ops as dve_ops
    from concourse.dve_ops import DveOp
    from concourse.dve_spec import C0, C1, Spec, Src0, Src1, lower, maxx, minn
    from concourse.dve_uop import DveOpSpec

    for op in dve_ops.OPS:
        if op.name == _OP_NAME:
            return op

    m1 = Src1 * C0
    m2 = Src1 * C1
    spec = Spec(
        body=minn(maxx(Src0 + m1, m1 - m2), m1 + m2) - m1,
        reference=_bfp_q5_reference,
    )
    row = dve_ops._CUSTOM_DVE_ROW_BASE + len(dve_ops.OPS)
    shas = {
        ver: DveOpSpec(
            name=_OP_NAME, opcode=row, uops=lower(spec, ver=ver), rd1_en=True
        ).sha(ver)
        for ver in ("v3", "v4")
    }
    op = DveOp(_OP_NAME, spec, subdim=False, uops_sha=shas)
    dve_ops.OPS.append(op)
    dve_ops.CUSTOM_DVE_SPECS[_OP_NAME] = spec
    dve_ops._SUB_OPCODE_FOR_NAME[_OP_NAME] = row
    return op


# ---------------------------------------------------------------------------
# Tile kernel (per core): x [4, 256, 3136] f32 -> y [4, 256, 3136] f32
# ---------------------------------------------------------------------------
B_PER_CORE = 4
C_CH = 256
HW = 3136          # 56*56 = N_BIG*PX_BIG + 64
PX_BIG = 512
N_BIG = 6
PX_REM = HW - N_BIG * PX_BIG   # 64
N_T = N_BIG + 1
N_C2 = PX_BIG // 128           # 128px chunks per tile
FD = N_C2 * 256                # xt free size
NJ = FD // 32


def bfp_tile_kernel(ctx, tc, y_ap, x_ap):
    nc = tc.nc
    op = _register_custom_op()

    const_pool = ctx.enter_context(tc.tile_pool(name="const", bufs=1))
    x_pool = ctx.enter_context(tc.tile_pool(name="xin", bufs=2))
    o_pool = ctx.enter_context(tc.tile_pool(name="osb", bufs=2))
    xt_pool = ctx.enter_context(tc.tile_pool(name="xt", bufs=3, space="PSUM"))
    on_pool = ctx.enter_context(tc.tile_pool(name="on", bufs=2, space="PSUM"))
    q_pool = ctx.enter_context(tc.tile_pool(name="q", bufs=3))
    m_pool = ctx.enter_context(tc.tile_pool(name="m", bufs=4))

    state = {}
    _idents = {}

    def ensure_idents():
        if _idents:
            return
        ident = const_pool.tile([128, 128], F32, name="ident")
        masks.make_identity(nc, ident[:])
        ident_bf = const_pool.tile([128, 128], BF16, name="ident_bf")
        masks.make_identity(nc, ident_bf[:])
        _idents["f32"] = ident
        _idents["bf16"] = ident_bf

    def emit_fwd(b, px0, npx, x_sb, out_sb):
        """Forward PE transposes of tile (b, px0..px0+npx) into PSUM."""
        if npx >= 128:
            nc2 = npx // 128
            xt = xt_pool.tile([128, nc2 * 256], F32, tag="xt", name=f"xt_{b}_{px0}")
            for c2 in range(nc2):
                for h in range(2):
                    seg = (c2 * 2 + h) * 128
                    nc.tensor.matmul(
                        xt[:, seg:seg + 128],
                        x_sb[:, h, px0 + 128 * c2:px0 + 128 * c2 + 128],
                        _idents["f32"][:, :],
                        is_transpose=True,
                    )
        else:
            xt = xt_pool.tile([64, 256], F32, tag="xt", name=f"xt_{b}_{px0}")
            for h in range(2):
                nc.tensor.matmul(
                    xt[:, h * 128:h * 128 + 128],
                    x_sb[:, h, px0:px0 + npx],
                    _idents["f32"][:, :],
                    is_transpose=True,
                )
        state[(b, px0)] = (xt, npx, out_sb)

    def emit_tail(b, px0):
        """Reduce + quantize + backward transposes + copy-out + store."""
        xt, npx, out_sb = state.pop((b, px0))
        big = npx >= 128
        parts = 128 if big else 64
        fd = xt.shape[1]
        nj = fd // 32
        xt3 = xt[:].rearrange("p (j k) -> p j k", k=32)

        mm = m_pool.tile([parts, nj], F32, tag="m" if big else "ms",
                         name=f"mm_{b}_{px0}")
        # split per PSUM bank (512 f32 cols) so each piece starts as soon as
        # its half of the forward transposes lands
        for lo in range(0, fd, 512):
            hi = min(lo + 512, fd)
            nc.vector.tensor_reduce(
                out=mm[:, lo // 32:hi // 32],
                in_=xt[:, lo:hi].rearrange("p (j k) -> p j k", k=32),
                axis=mybir.AxisListType.X,
                op=mybir.AluOpType.max, apply_absolute_value=True,
            )
        s0 = m_pool.tile([parts, nj], F32, tag="s0" if big else "s0s",
                         name=f"s0_{b}_{px0}")
        nc.vector.tensor_scalar(
            out=s0[:].bitcast(I32), in0=mm[:].bitcast(I32),
            scalar1=23, scalar2=23,
            op0=mybir.AluOpType.logical_shift_right,
            op1=mybir.AluOpType.logical_shift_left,
        )
        q = q_pool.tile([parts, nj * 32], BF16, tag="q", name=f"q_{b}_{px0}")
        nc.vector._custom_dve(
            op,
            out=q[:].rearrange("p (j k) -> p j k", k=32),
            in0=xt3,
            in1=s0[:].unsqueeze(-1).broadcast_to([parts, nj, 32]),
            s0=786432.0, s1=1.9375,
        )

        if big:
            nc2 = npx // 128
            on = on_pool.tile([128, fd], BF16, tag="on", name=f"on_{b}_{px0}")
            for c2 in range(nc2):
                for h in range(2):
                    seg = (c2 * 2 + h) * 128
                    nc.tensor.matmul(
                        on[:, seg:seg + 128],
                        q[:, 256 * c2 + 128 * h:256 * c2 + 128 * h + 128],
                        _idents["bf16"][:, :],
                        is_transpose=True,
                    )
            dst = out_sb[:, :, px0:px0 + npx].rearrange(
                "p h (c k) -> p c h k", k=128)
            nc.scalar.activation(dst, on[:], mybir.ActivationFunctionType.Copy)
        else:
            on = on_pool.tile([128, 128], BF16, tag="on", name=f"on_{b}_{px0}")
            for h in range(2):
                nc.tensor.matmul(
                    on[:, h * npx:(h + 1) * npx],
                    q[:, h * 128:h * 128 + 128],
                    _idents["bf16"][:64, :64],
                    is_transpose=True,
                )
            nc.scalar.activation(
                out_sb[:, :, px0:px0 + npx], on[:],
                mybir.ActivationFunctionType.Copy,
            )
        nc.scalar.dma_start(
            out=y_ap[b].rearrange("(h p) w -> p h w", p=128)[
                :, :, px0:px0 + npx],
            in_=out_sb[:, :, px0:px0 + npx],
        )

    # Software-pipelined emission: fwd transposes run ahead of each tile's
    # tail so the in-order PE queue interleaves them, input chunks are DMA'd
    # per tile with a lead, and batch 0 ramps in with small tiles so the
    # first chain starts as early as possible.
    full = [PX_BIG] * N_BIG + [PX_REM]
    jobs = []
    for b in range(B_PER_CORE):
        px0 = 0
        for npx in full:
            jobs.append((b, px0, npx))
            px0 += npx
    x_sbs, out_sbs = {}, {}

    def emit_in_chunk(b, px0, npx):
        if px0 == 0:
            x_sbs[b] = x_pool.tile([128, 2, HW], F32, tag="x", name=f"x_sb{b}")
            out_sbs[b] = o_pool.tile([128, 2, HW], F32, tag="o", name=f"out_sb{b}")
        xr = x_ap[b].rearrange("(h p) w -> p h w", p=128)
        nc.sync.dma_start(out=x_sbs[b][:, :, px0:px0 + npx],
                          in_=xr[:, :, px0:px0 + npx])

    prefetch = 0
    LAG = 2
    ensure_idents()
    for i, (b, px0, npx) in enumerate(jobs):
        while prefetch < len(jobs) and prefetch <= i + 2:
            emit_in_chunk(*jobs[prefetch])
            prefetch += 1
        emit_fwd(b, px0, npx, x_sbs[b], out_sbs[b])
        if i >= LAG:
            emit_tail(*jobs[i - LAG][:2])
    for j in jobs[len(jobs) - LAG:]:
        emit_tail(*j[:2])


# ---------------------------------------------------------------------------
# Build + run
# ---------------------------------------------------------------------------
_CACHED = {}


def build_bass(n_cores=8):
    from contextlib import ExitStack

    nc = bacc.Bacc(
        "TRN2",
        target_bir_lowering=False,
        debug=False,
        enable_asserts=False,
        num_devices=n_cores,
    )
    x = nc.dram_tensor("activations", [B_PER_CORE, C_CH, HW], F32,
                       kind="ExternalInput").ap()
    y = nc.dram_tensor("out", [B_PER_CORE, C_CH, HW], F32,
                       kind="ExternalOutput").ap()
    with tile.TileContext(nc) as tc:
        with ExitStack() as ctx:
            bfp_tile_kernel(ctx, tc, y, x)
    nc.compile()
    return nc


def kernel(activations: np.ndarray) -> np.ndarray:
    x = np.ascontiguousarray(np.asarray(activations), dtype=np.float32)
    B, C, H, W = x.shape            # [32, 256, 56, 56]
    n_cores = 8
    bpc = B // n_cores              # 4
    xs = x.reshape(n_cores, bpc, C, H * W)
    in_maps = [{"activations": np.ascontiguousarray(xs[c])} for c in range(n_cores)]

    if "nc" not in _CACHED:
        _CACHED["nc"] = build_bass(n_cores)
    nc = _CACHED["nc"]

    res = run_bass_kernel_spmd(nc, in_maps, core_ids=list(range(n_cores)))
    out = np.stack([res.results[c]["out"] for c in range(n_cores)])
    return out.reshape(B, C, H, W).astype(np.float32, copy=False)



# revision 4
# speedup vs baseline: 1.0314x; 1.0314x over previous
"""Trainium2 Bass kernel: BFP (block-floating-point) activation quantization.

Reference semantics (input NCHW [32, 256, 56, 56] f32):
  per (batch, pixel), channels grouped in blocks of 32:
    maxabs = max |x| over the block
    e      = floor(log2(maxabs))          (guard zero blocks)
    s      = 2^(e-4)                      (5-bit mantissa, QMAX = 31)
    out    = clip(round_half_even(x / s), -31, 31) * s    (0 if maxabs == 0)

Implementation (bit-exact in fp32, validated against the reference):
  s0 = 2^e is extracted by masking the exponent bits of maxabs.  The whole
  round+clip+rescale collapses into one fused DVE op using magic-number
  rounding in the C = 1.5*2^23 * s domain:
      C  = s0 * 786432.0        (= 1.5*2^23 * 2^-4 * s0 = magic * s)
      m  = s0 * 1.9375          (= 31 * s)
      out = min(max(x + C, C - m), C + m) - C
  Every step is exact in fp32: the x + C addition performs the
  round-half-even at ULP = s, the clip bounds and the final subtraction are
  exact multiples of s in the same binade.  The outputs are +-q * 2^(e-4)
  with q <= 31 (5 significant bits), so they are exactly representable in
  bf16 — the DRAM output is bf16 (half the store traffic) and the host
  widens it back to f32 losslessly.

Layout: channels live on SBUF partitions after the natural NCHW DMA, but the
block reduction needs channels along the free dim, so tiles are transposed
through the tensor engine in 128x128 chunks, processed in the
pixel-on-partition layout, and transposed back (in bf16, half cost).  The
batch and pixel dims are flattened (valid inside SBUF, where b and w are
adjacent) into one 12544-px axis tiled in uniform 512-px units, so there are
no 64-px runt tiles.  Input loads are 16 big quarter-batch DMAs on the sync
queue; output stores are 8 half-batch bf16 DMAs on the scalar queue, so
loads and stores stream concurrently with compute.

Sharding: batch 32 -> 4 per core across 8 NeuronCores; no cross-core comms.
"""

import numpy as np

import concourse.bass as bass
import concourse.mybir as mybir
from concourse import bacc, masks, tile
from concourse.bass_utils import run_bass_kernel_spmd

F32 = mybir.dt.float32
BF16 = mybir.dt.bfloat16
I32 = mybir.dt.int32

# If True, the custom DVE op takes raw maxabs as in1 and masks the exponent
# bits internally (BITWISE_AND with +Inf synthesized via MaxNeg*MaxNeg).
# If False, a separate tensor_scalar shift pair extracts s0 first.
FUSED_EXP = False

# ---------------------------------------------------------------------------
# Custom DVE ops: the entire quantize in one 1x pass.
# ---------------------------------------------------------------------------
_OP_NAME_SEP = "BFP_Q5_ANT"     # in1 = s0 (pre-masked exponent)
_OP_NAME_FUSED = "BFP_Q5F_ANT"  # in1 = maxabs (mask folded into the op)


def _bfp_q5_reference(in0, in1, s0, s1, imm2):
    in0 = np.asarray(in0, np.float32)
    in1 = np.asarray(in1, np.float32).reshape(in0.shape)
    c = (in1 * np.float32(s0)).astype(np.float32)
    m = (in1 * np.float32(s1)).astype(np.float32)
    u = (in0 + c).astype(np.float32)
    v = np.minimum(np.maximum(u, (c - m).astype(np.float32)),
                   (c + m).astype(np.float32)).astype(np.float32)
    return (v - c).astype(np.float32)


def _bfp_q5f_reference(in0, in1, s0, s1, imm2):
    in1 = np.asarray(in1, np.float32).reshape(np.asarray(in0).shape)
    s0f = (in1.view(np.uint32) & np.uint32(0x7F800000)).view(np.float32)
    return _bfp_q5_reference(in0, s0f, s0, s1, imm2)


def _register_custom_op():
    import concourse.dve_ops as dve_ops
    from concourse.dve_ops import DveOp
    from concourse.dve_spec import (
        C0, C1, MaxNeg, Spec, Src0, Src1, lower, maxx, minn,
    )
    from concourse.dve_spec import AluOp, Bin
    from concourse.dve_uop import DveOpSpec

    name = _OP_NAME_FUSED if FUSED_EXP else _OP_NAME_SEP
    for op in dve_ops.OPS:
        if op.name == name:
            return op

    if FUSED_EXP:
        # +Inf bit pattern (0x7f800000) synthesized by overflowing
        # MaxNeg*MaxNeg; stream-invariant, so it lowers to a latch.
        inf = Bin(AluOp.MULTIPLY, MaxNeg, MaxNeg)
        s0f = Bin(AluOp.BITWISE_AND, Src1, inf)
        ref = _bfp_q5f_reference
    else:
        s0f = Src1
        ref = _bfp_q5_reference
    # Clip bounds as single multiplies: C0-C1 = 786430.0625 and
    # C0+C1 = 786433.9375 are exact in fp32 (24 significant bits), and as
    # stream-invariant expressions they hoist to latches (no body stages).
    m1 = s0f * C0
    lo = s0f * (C0 - C1)
    hi = s0f * (C0 + C1)
    spec = Spec(
        body=minn(maxx(Src0 + m1, lo), hi) - m1,
        reference=ref,
    )
    row = dve_ops._CUSTOM_DVE_ROW_BASE + len(dve_ops.OPS)
    shas = {
        ver: DveOpSpec(
            name=name, opcode=row, uops=lower(spec, ver=ver), rd1_en=True
        ).sha(ver)
        for ver in ("v3", "v4")
    }
    op = DveOp(name, spec, subdim=False, uops_sha=shas)
    dve_ops.OPS.append(op)
    dve_ops.CUSTOM_DVE_SPECS[name] = spec
    dve_ops._SUB_OPCODE_FOR_NAME[name] = row
    return op


# ---------------------------------------------------------------------------
# Tile kernel (per core): x [4, 256, 3136] f32 -> y [4, 256, 3136] bf16
# ---------------------------------------------------------------------------
B_PER_CORE = 4
C_CH = 256
HW = 3136                      # 56*56
PX_TOTAL = B_PER_CORE * HW     # 12544 = 24*512 + 256
PX_UNIT = 512
N_FULL = PX_TOTAL // PX_UNIT   # 24
PX_REM = PX_TOTAL - N_FULL * PX_UNIT  # 256
LD_CHUNK = HW // 4             # 784 px per load DMA (16 loads)
ST_CHUNK = HW // 2             # 1568 px per store DMA (8 stores)


def bfp_tile_kernel(ctx, tc, y_ap, x_ap):
    nc = tc.nc
    op = _register_custom_op()

    const_pool = ctx.enter_context(tc.tile_pool(name="const", bufs=1))
    io_pool = ctx.enter_context(tc.tile_pool(name="io", bufs=1))
    xt_pool = ctx.enter_context(tc.tile_pool(name="xt", bufs=3, space="PSUM"))
    on_pool = ctx.enter_context(tc.tile_pool(name="on", bufs=2, space="PSUM"))
    q_pool = ctx.enter_context(tc.tile_pool(name="q", bufs=3))
    m_pool = ctx.enter_context(tc.tile_pool(name="m", bufs=4))

    ident = const_pool.tile([128, 128], F32, name="ident")
    masks.make_identity(nc, ident[:])
    ident_bf = const_pool.tile([128, 128], BF16, name="ident_bf")
    masks.make_identity(nc, ident_bf[:])

    # Persistent whole-core buffers; free layout [h, b, w] so (b w) flattens.
    x_sb = io_pool.tile([128, 2, B_PER_CORE, HW], F32, name="x_sb")
    out_sb = io_pool.tile([128, 2, B_PER_CORE, HW], BF16, name="out_sb")
    x_flat = x_sb[:].rearrange("p h b w -> p h (b w)")
    o_flat = out_sb[:].rearrange("p h b w -> p h (b w)")

    # ---- input loads: 16 quarter-batch DMAs on the sync queue ----
    for b in range(B_PER_CORE):
        xr = x_ap[b].rearrange("(h p) w -> p h w", p=128)
        for qtr in range(4):
            lo, hi = qtr * LD_CHUNK, (qtr + 1) * LD_CHUNK
            nc.sync.dma_start(out=x_sb[:, :, b, lo:hi], in_=xr[:, :, lo:hi])

    state = {}

    def emit_fwd(u, px0, npx):
        nc2 = npx // 128
        xt = xt_pool.tile([128, nc2 * 256], F32, tag="xt", name=f"xt_{u}")
        for c2 in range(nc2):
            for h in range(2):
                seg = (c2 * 2 + h) * 128
                nc.tensor.matmul(
                    xt[:, seg:seg + 128],
                    x_flat[:, h, px0 + 128 * c2:px0 + 128 * c2 + 128],
                    ident[:, :],
                    is_transpose=True,
                )
        state[u] = (xt, px0, npx)

    def emit_tail(u):
        xt, px0, npx = state.pop(u)
        nc2 = npx // 128
        fd = nc2 * 256
        nj = fd // 32
        xt3 = xt[:].rearrange("p (j k) -> p j k", k=32)

        mm = m_pool.tile([128, nj], F32, tag="m", name=f"mm_{u}")
        nc.vector.tensor_reduce(
            out=mm[:, :nj],
            in_=xt3,
            axis=mybir.AxisListType.X,
            op=mybir.AluOpType.max, apply_absolute_value=True,
        )
        if FUSED_EXP:
            s_in = mm
        else:
            s_in = m_pool.tile([128, nj], F32, tag="s0", name=f"s0_{u}")
            nc.vector.tensor_scalar(
                out=s_in[:, :nj].bitcast(I32), in0=mm[:, :nj].bitcast(I32),
                scalar1=23, scalar2=23,
                op0=mybir.AluOpType.logical_shift_right,
                op1=mybir.AluOpType.logical_shift_left,
            )
        q = q_pool.tile([128, fd], BF16, tag="q", name=f"q_{u}")
        nc.vector._custom_dve(
            op,
            out=q[:].rearrange("p (j k) -> p j k", k=32),
            in0=xt3,
            in1=s_in[:, :nj].unsqueeze(-1).broadcast_to([128, nj, 32]),
            s0=786432.0, s1=1.9375,
        )

        on = on_pool.tile([128, fd], BF16, tag="on", name=f"on_{u}")
        for c2 in range(nc2):
            for h in range(2):
                seg = (c2 * 2 + h) * 128
                nc.tensor.matmul(
                    on[:, seg:seg + 128],
                    q[:, seg:seg + 128],
                    ident_bf[:, :],
                    is_transpose=True,
                )
        dst = o_flat[:, :, px0:px0 + npx].rearrange("p h (c k) -> p c h k", k=128)
        nc.scalar.activation(dst, on[:], mybir.ActivationFunctionType.Copy)

    # ---- store emission: half-batch bf16 DMAs on the scalar queue ----
    def emit_store(hb):
        b, half = divmod(hb, 2)
        lo, hi = half * ST_CHUNK, (half + 1) * ST_CHUNK
        yr = y_ap[b].rearrange("(h p) w -> p h w", p=128)
        nc.scalar.dma_start(out=yr[:, :, lo:hi], in_=out_sb[:, :, b, lo:hi])

    units = [(u, u * PX_UNIT, PX_UNIT) for u in range(N_FULL)]
    if PX_REM:
        units.append((N_FULL, N_FULL * PX_UNIT, PX_REM))
    # store hb becomes ready once the unit covering its last pixel is done
    store_after = {}
    for hb in range(2 * B_PER_CORE):
        last_px = (hb + 1) * ST_CHUNK - 1
        store_after.setdefault(min(last_px // PX_UNIT, len(units) - 1), []).append(hb)

    LAG = 2
    done = []
    for i, (u, px0, npx) in enumerate(units):
        emit_fwd(u, px0, npx)
        if i >= LAG:
            emit_tail(units[i - LAG][0])
            done.append(units[i - LAG][0])
            for hb in store_after.get(done[-1], []):
                emit_store(hb)
    for u, _, _ in units[len(units) - LAG:]:
        emit_tail(u)
        for hb in store_after.get(u, []):
            emit_store(hb)


# ---------------------------------------------------------------------------
# Build + run
# ---------------------------------------------------------------------------
_CACHED = {}


def build_bass(n_cores=8):
    from contextlib import ExitStack

    nc = bacc.Bacc(
        "TRN2",
        target_bir_lowering=False,
        debug=False,
        enable_asserts=False,
        num_devices=n_cores,
    )
    x = nc.dram_tensor("activations", [B_PER_CORE, C_CH, HW], F32,
                       kind="ExternalInput").ap()
    y = nc.dram_tensor("out", [B_PER_CORE, C_CH, HW], BF16,
                       kind="ExternalOutput").ap()
    with tile.TileContext(nc) as tc:
        with ExitStack() as ctx:
            bfp_tile_kernel(ctx, tc, y, x)
    nc.compile()
    return nc


def kernel(activations: np.ndarray) -> np.ndarray:
    x = np.ascontiguousarray(np.asarray(activations), dtype=np.float32)
    B, C, H, W = x.shape            # [32, 256, 56, 56]
    n_cores = 8
    bpc = B // n_cores              # 4
    xs = x.reshape(n_cores, bpc, C, H * W)
    in_maps = [{"activations": np.ascontiguousarray(xs[c])} for c in range(n_cores)]

    if "nc" not in _CACHED:
        _CACHED["nc"] = build_bass(n_cores)
    nc = _CACHED["nc"]

    res = run_bass_kernel_spmd(nc, in_maps, core_ids=list(range(n_cores)))
    outs = []
    for c in range(n_cores):
        o = np.asarray(res.results[c]["out"])
        if o.dtype != np.float32:
            o = o.astype(np.float32)   # bf16 -> f32 widen, lossless
        outs.append(o)
    out = np.stack(outs)
    return out.reshape(B, C, H, W)


# revision 9
# speedup vs baseline: 1.0485x; 1.0166x over previous
"""Trainium2 Bass kernel: BFP (block-floating-point) activation quantization.

Reference semantics (input NCHW [32, 256, 56, 56] f32):
  per (batch, pixel), channels grouped in blocks of 32:
    maxabs = max |x| over the block
    e      = floor(log2(maxabs))          (guard zero blocks)
    s      = 2^(e-4)                      (5-bit mantissa, QMAX = 31)
    out    = clip(round_half_even(x / s), -31, 31) * s    (0 if maxabs == 0)

Implementation (bit-exact in fp32, validated against the reference):
  s0 = 2^e is extracted by masking the exponent bits of maxabs.  The whole
  round+clip+rescale collapses into one fused DVE op using magic-number
  rounding in the C = 1.5*2^23 * s domain:
      C  = s0 * 786432.0        (= 1.5*2^23 * 2^-4 * s0 = magic * s)
      m  = s0 * 1.9375          (= 31 * s)
      out = min(max(x + C, C - m), C + m) - C
  Every step is exact in fp32: the x + C addition performs the
  round-half-even at ULP = s, the clip bounds and the final subtraction are
  exact multiples of s in the same binade.  The outputs are +-q * 2^(e-4)
  with q <= 31 (5 significant bits), so they are exactly representable in
  bf16 — the DRAM output is bf16 (half the store traffic) and the host
  widens it back to f32 losslessly.

Layout: channels live on SBUF partitions after the natural NCHW DMA, but the
block reduction needs channels along the free dim, so tiles are transposed
through the tensor engine in 128x128 chunks, processed in the
pixel-on-partition layout, and transposed back (in bf16, half cost).  The
batch and pixel dims are flattened (valid inside SBUF, where b and w are
adjacent) into one 12544-px axis tiled in uniform 512-px units, so there are
no 64-px runt tiles.  Input loads are 16 big quarter-batch DMAs on the sync
queue; output stores are 8 half-batch bf16 DMAs on the scalar queue, so
loads and stores stream concurrently with compute.

Sharding: batch 32 -> 4 per core across 8 NeuronCores; no cross-core comms.
"""

import numpy as np

import concourse.bass as bass
import concourse.mybir as mybir
from concourse import bacc, masks, tile
from concourse.bass_utils import run_bass_kernel_spmd

F32 = mybir.dt.float32
BF16 = mybir.dt.bfloat16
I32 = mybir.dt.int32

# If True, the custom DVE op takes raw maxabs as in1 and masks the exponent
# bits internally (BITWISE_AND with +Inf synthesized via MaxNeg*MaxNeg).
# If False, a separate tensor_scalar shift pair extracts s0 first.
FUSED_EXP = False

# ---------------------------------------------------------------------------
# Custom DVE ops: the entire quantize in one 1x pass.
# ---------------------------------------------------------------------------
_OP_NAME_SEP = "BFP_Q5_ANT"     # in1 = s0 (pre-masked exponent)
_OP_NAME_FUSED = "BFP_Q5F_ANT"  # in1 = maxabs (mask folded into the op)


def _bfp_q5_reference(in0, in1, s0, s1, imm2):
    in0 = np.asarray(in0, np.float32)
    in1 = np.asarray(in1, np.float32).reshape(in0.shape)
    c = (in1 * np.float32(s0)).astype(np.float32)
    m = (in1 * np.float32(s1)).astype(np.float32)
    u = (in0 + c).astype(np.float32)
    v = np.minimum(np.maximum(u, (c - m).astype(np.float32)),
                   (c + m).astype(np.float32)).astype(np.float32)
    return (v - c).astype(np.float32)


def _bfp_q5f_reference(in0, in1, s0, s1, imm2):
    in1 = np.asarray(in1, np.float32).reshape(np.asarray(in0).shape)
    s0f = (in1.view(np.uint32) & np.uint32(0x7F800000)).view(np.float32)
    return _bfp_q5_reference(in0, s0f, s0, s1, imm2)


def _register_custom_op():
    import concourse.dve_ops as dve_ops
    from concourse.dve_ops import DveOp
    from concourse.dve_spec import (
        C0, C1, MaxNeg, Spec, Src0, Src1, lower, maxx, minn,
    )
    from concourse.dve_spec import AluOp, Bin
    from concourse.dve_uop import DveOpSpec

    name = _OP_NAME_FUSED if FUSED_EXP else _OP_NAME_SEP
    for op in dve_ops.OPS:
        if op.name == name:
            return op

    if FUSED_EXP:
        # +Inf bit pattern (0x7f800000) synthesized by overflowing
        # MaxNeg*MaxNeg; stream-invariant, so it lowers to a latch.
        inf = Bin(AluOp.MULTIPLY, MaxNeg, MaxNeg)
        s0f = Bin(AluOp.BITWISE_AND, Src1, inf)
        ref = _bfp_q5f_reference
    else:
        s0f = Src1
        ref = _bfp_q5_reference
    # Clip bounds as single multiplies: C0-C1 = 786430.0625 and
    # C0+C1 = 786433.9375 are exact in fp32 (24 significant bits), and as
    # stream-invariant expressions they hoist to latches (no body stages).
    m1 = s0f * C0
    lo = s0f * (C0 - C1)
    hi = s0f * (C0 + C1)
    spec = Spec(
        body=minn(maxx(Src0 + m1, lo), hi) - m1,
        reference=ref,
    )
    row = dve_ops._CUSTOM_DVE_ROW_BASE + len(dve_ops.OPS)
    shas = {
        ver: DveOpSpec(
            name=name, opcode=row, uops=lower(spec, ver=ver), rd1_en=True
        ).sha(ver)
        for ver in ("v3", "v4")
    }
    op = DveOp(name, spec, subdim=False, uops_sha=shas)
    dve_ops.OPS.append(op)
    dve_ops.CUSTOM_DVE_SPECS[name] = spec
    dve_ops._SUB_OPCODE_FOR_NAME[name] = row
    return op


# ---------------------------------------------------------------------------
# Tile kernel (per core): x [4, 256, 3136] f32 -> y [4, 256, 3136] bf16
# ---------------------------------------------------------------------------
B_PER_CORE = 4
C_CH = 256
HW = 3136                      # 56*56
PX_TOTAL = B_PER_CORE * HW     # 12544 = 24*512 + 256
PX_UNIT = 512
N_FULL = PX_TOTAL // PX_UNIT   # 24
PX_REM = PX_TOTAL - N_FULL * PX_UNIT  # 256
LD_CHUNK = HW // 4             # 784 px per load DMA (16 loads)
ST_CHUNK = HW // 2             # 1568 px per store DMA (8 stores)


def bfp_tile_kernel(ctx, tc, y_ap, x_ap):
    nc = tc.nc
    op = _register_custom_op()

    const_pool = ctx.enter_context(tc.tile_pool(name="const", bufs=1))
    io_pool = ctx.enter_context(tc.tile_pool(name="io", bufs=1))
    xt_pool = ctx.enter_context(tc.tile_pool(name="xt", bufs=3, space="PSUM"))
    on_pool = ctx.enter_context(tc.tile_pool(name="on", bufs=2, space="PSUM"))
    q_pool = ctx.enter_context(tc.tile_pool(name="q", bufs=3))
    m_pool = ctx.enter_context(tc.tile_pool(name="m", bufs=4))

    # Persistent whole-core buffers; free layout [h, b, w] so (b w) flattens.
    x_sb = io_pool.tile([128, 2, B_PER_CORE, HW], F32, name="x_sb")
    out_sb = io_pool.tile([128, 2, B_PER_CORE, HW], BF16, name="out_sb")
    x_flat = x_sb[:].rearrange("p h b w -> p h (b w)")
    o_flat = out_sb[:].rearrange("p h b w -> p h (b w)")

    # ---- input loads first (before ident setup) so data streams during
    # setup.  First chunk is small so unit 0 can start ASAP.
    for b in range(B_PER_CORE):
        xr = x_ap[b].rearrange("(h p) w -> p h w", p=128)
        for qtr in range(4):
            lo, hi = qtr * LD_CHUNK, (qtr + 1) * LD_CHUNK
            if b == 0 and qtr == 0:
                nc.sync.dma_start(out=x_sb[:, :, b, 0:256], in_=xr[:, :, 0:256])
                nc.sync.dma_start(out=x_sb[:, :, b, 256:hi], in_=xr[:, :, 256:hi])
            else:
                nc.sync.dma_start(out=x_sb[:, :, b, lo:hi], in_=xr[:, :, lo:hi])

    ident = const_pool.tile([128, 128], F32, name="ident")
    masks.make_identity(nc, ident[:])
    ident_bf = const_pool.tile([128, 128], BF16, name="ident_bf")
    masks.make_identity(nc, ident_bf[:])

    state = {}

    def emit_fwd(u, px0, npx):
        nc2 = npx // 128
        xt = xt_pool.tile([128, nc2 * 256], F32, tag="xt", name=f"xt_{u}")
        for c2 in range(nc2):
            for h in range(2):
                seg = (c2 * 2 + h) * 128
                nc.tensor.matmul(
                    xt[:, seg:seg + 128],
                    x_flat[:, h, px0 + 128 * c2:px0 + 128 * c2 + 128],
                    ident[:, :],
                    is_transpose=True,
                )
        state[u] = (xt, px0, npx)

    def emit_reduce(u):
        """maxabs reduce on DVE, then exponent extract on (otherwise idle)
        GpSimd so the DVE only runs the two big ops per unit."""
        xt, px0, npx = state[u]
        nj = (npx // 128) * 8
        xt3 = xt[:].rearrange("p (j k) -> p j k", k=32)
        mm = m_pool.tile([128, nj], F32, tag="m", name=f"mm_{u}")
        nc.vector.tensor_reduce(
            out=mm[:, :nj],
            in_=xt3,
            axis=mybir.AxisListType.X,
            op=mybir.AluOpType.max, apply_absolute_value=True,
        )
        s0 = m_pool.tile([128, nj], F32, tag="s0", name=f"s0_{u}")
        nc.vector.tensor_scalar(
            out=s0[:, :nj].bitcast(I32), in0=mm[:, :nj].bitcast(I32),
            scalar1=23, scalar2=23,
            op0=mybir.AluOpType.logical_shift_right,
            op1=mybir.AluOpType.logical_shift_left,
        )
        state[u] = (xt, px0, npx, s0)

    def emit_quant(u):
        xt, px0, npx, s0 = state.pop(u)
        nc2 = npx // 128
        fd = nc2 * 256
        nj = fd // 32
        q = q_pool.tile([128, fd], BF16, tag="q", name=f"q_{u}")
        nc.vector._custom_dve(
            op,
            out=q[:].rearrange("p (j k) -> p j k", k=32),
            in0=xt[:].rearrange("p (j k) -> p j k", k=32),
            in1=s0[:, :nj].unsqueeze(-1).broadcast_to([128, nj, 32]),
            s0=786432.0, s1=1.9375,
        )

        on = on_pool.tile([128, fd], BF16, tag="on", name=f"on_{u}")
        # back-transpose + copy-out in halves so the scalar-engine copy of
        # the first half overlaps the second half's transposes
        half = max(nc2 // 2, 1)
        for c2 in range(nc2):
            for h in range(2):
                seg = (c2 * 2 + h) * 128
                nc.tensor.matmul(
                    on[:, seg:seg + 128],
                    q[:, seg:seg + 128],
                    ident_bf[:, :],
                    is_transpose=True,
                )
            if c2 + 1 == half or c2 + 1 == nc2:
                c0 = 0 if c2 + 1 == half else half
                if c0 == 0 and c2 + 1 == nc2:
                    c0 = 0  # single chunk unit
                pl, ph = px0 + c0 * 128, px0 + (c2 + 1) * 128
                dst = o_flat[:, :, pl:ph].rearrange("p h (c k) -> p c h k", k=128)
                nc.scalar.activation(
                    dst, on[:, c0 * 256:(c2 + 1) * 256],
                    mybir.ActivationFunctionType.Copy,
                )

    # ---- store emission: half-batch bf16 DMAs on the scalar queue; the
    # last store is split in two so the drain tail is short ----
    def emit_store(hb):
        b, half = divmod(hb, 2)
        lo, hi = half * ST_CHUNK, (half + 1) * ST_CHUNK
        yr = y_ap[b].rearrange("(h p) w -> p h w", p=128)
        if hb == 2 * B_PER_CORE - 1:
            mid = (lo + hi) // 2
            nc.scalar.dma_start(out=yr[:, :, lo:mid], in_=out_sb[:, :, b, lo:mid])
            nc.scalar.dma_start(out=yr[:, :, mid:hi], in_=out_sb[:, :, b, mid:hi])
        else:
            nc.scalar.dma_start(out=yr[:, :, lo:hi], in_=out_sb[:, :, b, lo:hi])

    units = [(u, u * PX_UNIT, PX_UNIT) for u in range(N_FULL)]
    if PX_REM:
        units.append((N_FULL, N_FULL * PX_UNIT, PX_REM))
    # store hb becomes ready once the unit covering its last pixel is done
    store_after = {}
    for hb in range(2 * B_PER_CORE):
        last_px = (hb + 1) * ST_CHUNK - 1
        store_after.setdefault(min(last_px // PX_UNIT, len(units) - 1), []).append(hb)

    # pipeline: fwd(i) | reduce(i-1) | quant(i-2)+back+copy+stores
    n = len(units)
    for i in range(n + 2):
        if i < n:
            emit_fwd(*units[i])
        if 1 <= i < n + 1:
            emit_reduce(units[i - 1][0])
        if i >= 2:
            u = units[i - 2][0]
            emit_quant(u)
            for hb in store_after.get(u, []):
                emit_store(hb)


# ---------------------------------------------------------------------------
# Build + run
# ---------------------------------------------------------------------------
_CACHED = {}


def build_bass(n_cores=8):
    from contextlib import ExitStack

    nc = bacc.Bacc(
        "TRN2",
        target_bir_lowering=False,
        debug=False,
        enable_asserts=False,
        num_devices=n_cores,
    )
    x = nc.dram_tensor("activations", [B_PER_CORE, C_CH, HW], F32,
                       kind="ExternalInput").ap()
    y = nc.dram_tensor("out", [B_PER_CORE, C_CH, HW], BF16,
                       kind="ExternalOutput").ap()
    with tile.TileContext(nc) as tc:
        with ExitStack() as ctx:
            bfp_tile_kernel(ctx, tc, y, x)
    nc.compile()
    return nc


def kernel(activations: np.ndarray) -> np.ndarray:
    x = np.ascontiguousarray(np.asarray(activations), dtype=np.float32)
    B, C, H, W = x.shape            # [32, 256, 56, 56]
    n_cores = 8
    bpc = B // n_cores              # 4
    xs = x.reshape(n_cores, bpc, C, H * W)
    in_maps = [{"activations": np.ascontiguousarray(xs[c])} for c in range(n_cores)]

    if "nc" not in _CACHED:
        _CACHED["nc"] = build_bass(n_cores)
    nc = _CACHED["nc"]

    res = run_bass_kernel_spmd(nc, in_maps, core_ids=list(range(n_cores)))
    outs = []
    for c in range(n_cores):
        o = np.asarray(res.results[c]["out"])
        if o.dtype != np.float32:
            o = o.astype(np.float32)   # bf16 -> f32 widen, lossless
        outs.append(o)
    out = np.stack(outs)
    return out.reshape(B, C, H, W)


# revision 12
# speedup vs baseline: 1.0603x; 1.0112x over previous
"""Trainium2 Bass kernel: BFP (block-floating-point) activation quantization.

Reference semantics (input NCHW [32, 256, 56, 56] f32):
  per (batch, pixel), channels grouped in blocks of 32:
    maxabs = max |x| over the block
    e      = floor(log2(maxabs))          (guard zero blocks)
    s      = 2^(e-4)                      (5-bit mantissa, QMAX = 31)
    out    = clip(round_half_even(x / s), -31, 31) * s    (0 if maxabs == 0)

Implementation (bit-exact in fp32, validated against the reference):
  s0 = 2^e is extracted by masking the exponent bits of maxabs.  The whole
  round+clip+rescale collapses into one fused DVE op using magic-number
  rounding in the C = 1.5*2^23 * s domain:
      C  = s0 * 786432.0        (= 1.5*2^23 * 2^-4 * s0 = magic * s)
      m  = s0 * 1.9375          (= 31 * s)
      out = min(max(x + C, C - m), C + m) - C
  Every step is exact in fp32: the x + C addition performs the
  round-half-even at ULP = s, the clip bounds and the final subtraction are
  exact multiples of s in the same binade.  The outputs are +-q * 2^(e-4)
  with q <= 31 (5 significant bits), so they are exactly representable in
  bf16 — the DRAM output is bf16 (half the store traffic) and the host
  widens it back to f32 losslessly.

Layout: channels live on SBUF partitions after the natural NCHW DMA, but the
block reduction needs channels along the free dim, so tiles are transposed
through the tensor engine in 128x128 chunks, processed in the
pixel-on-partition layout, and transposed back (in bf16, half cost).  The
batch and pixel dims are flattened (valid inside SBUF, where b and w are
adjacent) into one 12544-px axis tiled in uniform 512-px units, so there are
no 64-px runt tiles.  Input loads are 16 big quarter-batch DMAs on the sync
queue; output stores are 8 half-batch bf16 DMAs on the scalar queue, so
loads and stores stream concurrently with compute.

Sharding: batch 32 -> 4 per core across 8 NeuronCores; no cross-core comms.
"""

import numpy as np

import concourse.bass as bass
import concourse.mybir as mybir
from concourse import bacc, masks, tile
from concourse.bass_utils import run_bass_kernel_spmd

F32 = mybir.dt.float32
BF16 = mybir.dt.bfloat16
I32 = mybir.dt.int32

# If True, the custom DVE op takes raw maxabs as in1 and masks the exponent
# bits internally (BITWISE_AND with +Inf synthesized via MaxNeg*MaxNeg).
# If False, a separate tensor_scalar shift pair extracts s0 first.
FUSED_EXP = False

# ---------------------------------------------------------------------------
# Custom DVE ops: the entire quantize in one 1x pass.
# ---------------------------------------------------------------------------
_OP_NAME_SEP = "BFP_Q5_ANT"     # in1 = s0 (pre-masked exponent)
_OP_NAME_FUSED = "BFP_Q5F_ANT"  # in1 = maxabs (mask folded into the op)


def _bfp_q5_reference(in0, in1, s0, s1, imm2):
    in0 = np.asarray(in0, np.float32)
    in1 = np.asarray(in1, np.float32).reshape(in0.shape)
    c = (in1 * np.float32(s0)).astype(np.float32)
    m = (in1 * np.float32(s1)).astype(np.float32)
    u = (in0 + c).astype(np.float32)
    v = np.minimum(np.maximum(u, (c - m).astype(np.float32)),
                   (c + m).astype(np.float32)).astype(np.float32)
    return (v - c).astype(np.float32)


def _bfp_q5f_reference(in0, in1, s0, s1, imm2):
    in1 = np.asarray(in1, np.float32).reshape(np.asarray(in0).shape)
    s0f = (in1.view(np.uint32) & np.uint32(0x7F800000)).view(np.float32)
    return _bfp_q5_reference(in0, s0f, s0, s1, imm2)


def _register_custom_op():
    import concourse.dve_ops as dve_ops
    from concourse.dve_ops import DveOp
    from concourse.dve_spec import (
        C0, C1, MaxNeg, Spec, Src0, Src1, lower, maxx, minn,
    )
    from concourse.dve_spec import AluOp, Bin
    from concourse.dve_uop import DveOpSpec

    name = _OP_NAME_FUSED if FUSED_EXP else _OP_NAME_SEP
    for op in dve_ops.OPS:
        if op.name == name:
            return op

    if FUSED_EXP:
        # +Inf bit pattern (0x7f800000) synthesized by overflowing
        # MaxNeg*MaxNeg; stream-invariant, so it lowers to a latch.
        inf = Bin(AluOp.MULTIPLY, MaxNeg, MaxNeg)
        s0f = Bin(AluOp.BITWISE_AND, Src1, inf)
        ref = _bfp_q5f_reference
    else:
        s0f = Src1
        ref = _bfp_q5_reference
    # Clip bounds as single multiplies: C0-C1 = 786430.0625 and
    # C0+C1 = 786433.9375 are exact in fp32 (24 significant bits), and as
    # stream-invariant expressions they hoist to latches (no body stages).
    m1 = s0f * C0
    lo = s0f * (C0 - C1)
    hi = s0f * (C0 + C1)
    spec = Spec(
        body=minn(maxx(Src0 + m1, lo), hi) - m1,
        reference=ref,
    )
    row = dve_ops._CUSTOM_DVE_ROW_BASE + len(dve_ops.OPS)
    shas = {
        ver: DveOpSpec(
            name=name, opcode=row, uops=lower(spec, ver=ver), rd1_en=True
        ).sha(ver)
        for ver in ("v3", "v4")
    }
    op = DveOp(name, spec, subdim=False, uops_sha=shas)
    dve_ops.OPS.append(op)
    dve_ops.CUSTOM_DVE_SPECS[name] = spec
    dve_ops._SUB_OPCODE_FOR_NAME[name] = row
    return op


# ---------------------------------------------------------------------------
# Tile kernel (per core): x [4, 256, 3136] f32 -> y [4, 256, 3136] bf16
# ---------------------------------------------------------------------------
B_PER_CORE = 4
C_CH = 256
HW = 3136                      # 56*56
PX_TOTAL = B_PER_CORE * HW     # 12544 = 24*512 + 256
PX_UNIT = 512
N_FULL = PX_TOTAL // PX_UNIT   # 24
PX_REM = PX_TOTAL - N_FULL * PX_UNIT  # 256
LD_CHUNK = HW // 4             # 784 px per load DMA (16 loads)
ST_CHUNK = HW // 2             # 1568 px per store DMA (8 stores)


def bfp_tile_kernel(ctx, tc, y_ap, x_ap):
    nc = tc.nc
    op = _register_custom_op()

    const_pool = ctx.enter_context(tc.tile_pool(name="const", bufs=1))
    io_pool = ctx.enter_context(tc.tile_pool(name="io", bufs=1))
    xt_pool = ctx.enter_context(tc.tile_pool(name="xt", bufs=3, space="PSUM"))
    on_pool = ctx.enter_context(tc.tile_pool(name="on", bufs=2, space="PSUM"))
    q_pool = ctx.enter_context(tc.tile_pool(name="q", bufs=3))
    m_pool = ctx.enter_context(tc.tile_pool(name="m", bufs=4))

    # Persistent whole-core buffers; free layout [h, b, w] so (b w) flattens.
    x_sb = io_pool.tile([128, 2, B_PER_CORE, HW], F32, name="x_sb")
    out_sb = io_pool.tile([128, 2, B_PER_CORE, HW], BF16, name="out_sb")
    x_flat = x_sb[:].rearrange("p h b w -> p h (b w)")
    o_flat = out_sb[:].rearrange("p h b w -> p h (b w)")

    # ---- input loads first (before ident setup) so data streams during
    # setup.  First chunk is small so unit 0 can start ASAP.
    for b in range(B_PER_CORE):
        xr = x_ap[b].rearrange("(h p) w -> p h w", p=128)
        for qtr in range(4):
            lo, hi = qtr * LD_CHUNK, (qtr + 1) * LD_CHUNK
            if b == 0 and qtr == 0:
                # unit 0's data, split across both HWDGE queues so the two
                # halves transfer in parallel during the ramp
                nc.sync.dma_start(out=x_sb[:, :, b, 0:256], in_=xr[:, :, 0:256])
                nc.scalar.dma_start(out=x_sb[:, :, b, 256:512], in_=xr[:, :, 256:512])
                nc.sync.dma_start(out=x_sb[:, :, b, 512:hi], in_=xr[:, :, 512:hi])
            else:
                nc.sync.dma_start(out=x_sb[:, :, b, lo:hi], in_=xr[:, :, lo:hi])

    ident = const_pool.tile([128, 128], F32, name="ident")
    masks.make_identity(nc, ident[:])
    ident_bf = const_pool.tile([128, 128], BF16, name="ident_bf")
    masks.make_identity(nc, ident_bf[:])

    state = {}

    def emit_fwd(u, px0, npx):
        nc2 = npx // 128
        xt = xt_pool.tile([128, nc2 * 256], F32, tag="xt", name=f"xt_{u}")
        for c2 in range(nc2):
            for h in range(2):
                seg = (c2 * 2 + h) * 128
                nc.tensor.matmul(
                    xt[:, seg:seg + 128],
                    x_flat[:, h, px0 + 128 * c2:px0 + 128 * c2 + 128],
                    ident[:, :],
                    is_transpose=True,
                )
        state[u] = (xt, px0, npx)

    def emit_reduce(u):
        """maxabs reduce on DVE, then exponent extract on (otherwise idle)
        GpSimd so the DVE only runs the two big ops per unit."""
        xt, px0, npx = state[u]
        nj = (npx // 128) * 8
        xt3 = xt[:].rearrange("p (j k) -> p j k", k=32)
        mm = m_pool.tile([128, nj], F32, tag="m", name=f"mm_{u}")
        nc.vector.tensor_reduce(
            out=mm[:, :nj],
            in_=xt3,
            axis=mybir.AxisListType.X,
            op=mybir.AluOpType.max, apply_absolute_value=True,
        )
        s0 = m_pool.tile([128, nj], F32, tag="s0", name=f"s0_{u}")
        nc.vector.tensor_scalar(
            out=s0[:, :nj].bitcast(I32), in0=mm[:, :nj].bitcast(I32),
            scalar1=0x7F800000, scalar2=None,
            op0=mybir.AluOpType.bitwise_and,
        )
        state[u] = (xt, px0, npx, s0)

    def emit_quant(u):
        xt, px0, npx, s0 = state.pop(u)
        nc2 = npx // 128
        fd = nc2 * 256
        nj = fd // 32
        q = q_pool.tile([128, fd], BF16, tag="q", name=f"q_{u}")
        nc.vector._custom_dve(
            op,
            out=q[:].rearrange("p (j k) -> p j k", k=32),
            in0=xt[:].rearrange("p (j k) -> p j k", k=32),
            in1=s0[:, :nj].unsqueeze(-1).broadcast_to([128, nj, 32]),
            s0=786432.0, s1=1.9375,
        )

        on = on_pool.tile([128, fd], BF16, tag="on", name=f"on_{u}")
        # back-transpose + copy-out in halves so the scalar-engine copy of
        # the first half overlaps the second half's transposes
        half = max(nc2 // 2, 1)
        for c2 in range(nc2):
            for h in range(2):
                seg = (c2 * 2 + h) * 128
                nc.tensor.matmul(
                    on[:, seg:seg + 128],
                    q[:, seg:seg + 128],
                    ident_bf[:, :],
                    is_transpose=True,
                )
            if c2 + 1 == half or c2 + 1 == nc2:
                c0 = 0 if c2 + 1 == half else half
                if c0 == 0 and c2 + 1 == nc2:
                    c0 = 0  # single chunk unit
                pl, ph = px0 + c0 * 128, px0 + (c2 + 1) * 128
                dst = o_flat[:, :, pl:ph].rearrange("p h (c k) -> p c h k", k=128)
                nc.scalar.activation(
                    dst, on[:, c0 * 256:(c2 + 1) * 256],
                    mybir.ActivationFunctionType.Copy,
                )

    # ---- store emission: half-batch bf16 DMAs on the scalar queue; the
    # last store is split in two so the drain tail is short ----
    def emit_store(hb):
        b, half = divmod(hb, 2)
        lo, hi = half * ST_CHUNK, (half + 1) * ST_CHUNK
        yr = y_ap[b].rearrange("(h p) w -> p h w", p=128)
        if hb == 2 * B_PER_CORE - 1:
            mid = (lo + hi) // 2
            nc.scalar.dma_start(out=yr[:, :, lo:mid], in_=out_sb[:, :, b, lo:mid])
            nc.scalar.dma_start(out=yr[:, :, mid:hi], in_=out_sb[:, :, b, mid:hi])
        else:
            nc.scalar.dma_start(out=yr[:, :, lo:hi], in_=out_sb[:, :, b, lo:hi])

    units = [(u, u * PX_UNIT, PX_UNIT) for u in range(N_FULL)]
    if PX_REM:
        units.append((N_FULL, N_FULL * PX_UNIT, PX_REM))
    # store hb becomes ready once the unit covering its last pixel is done
    store_after = {}
    for hb in range(2 * B_PER_CORE):
        last_px = (hb + 1) * ST_CHUNK - 1
        store_after.setdefault(min(last_px // PX_UNIT, len(units) - 1), []).append(hb)

    # pipeline: fwd(i) | reduce(i-1) | quant(i-2)+back+copy+stores
    n = len(units)
    for i in range(n + 2):
        if i < n:
            emit_fwd(*units[i])
        if 1 <= i < n + 1:
            emit_reduce(units[i - 1][0])
        if i >= 2:
            u = units[i - 2][0]
            emit_quant(u)
            for hb in store_after.get(u, []):
                emit_store(hb)


# ---------------------------------------------------------------------------
# Build + run
# ---------------------------------------------------------------------------
_CACHED = {}


def build_bass(n_cores=8):
    from contextlib import ExitStack

    nc = bacc.Bacc(
        "TRN2",
        target_bir_lowering=False,
        debug=False,
        enable_asserts=False,
        num_devices=n_cores,
    )
    x = nc.dram_tensor("activations", [B_PER_CORE, C_CH, HW], F32,
                       kind="ExternalInput").ap()
    y = nc.dram_tensor("out", [B_PER_CORE, C_CH, HW], BF16,
                       kind="ExternalOutput").ap()
    with tile.TileContext(nc) as tc:
        with ExitStack() as ctx:
            bfp_tile_kernel(ctx, tc, y, x)
    nc.compile()
    return nc


def kernel(activations: np.ndarray) -> np.ndarray:
    x = np.ascontiguousarray(np.asarray(activations), dtype=np.float32)
    B, C, H, W = x.shape            # [32, 256, 56, 56]
    n_cores = 8
    bpc = B // n_cores              # 4
    xs = x.reshape(n_cores, bpc, C, H * W)
    in_maps = [{"activations": np.ascontiguousarray(xs[c])} for c in range(n_cores)]

    if "nc" not in _CACHED:
        _CACHED["nc"] = build_bass(n_cores)
    nc = _CACHED["nc"]

    res = run_bass_kernel_spmd(nc, in_maps, core_ids=list(range(n_cores)))
    outs = []
    for c in range(n_cores):
        o = np.asarray(res.results[c]["out"])
        if o.dtype != np.float32:
            o = o.astype(np.float32)   # bf16 -> f32 widen, lossless
        outs.append(o)
    out = np.stack(outs)
    return out.reshape(B, C, H, W)
